# revision 39
# baseline (speedup 1.0000x reference)
"""Block-diagonal compress kernel: out = blockdiag(A) @ W @ blockdiag(B).

Shapes (full): W [8192, 8192] f32, A_blocks [128, 64, 64], B_blocks [128, 64, 64].
Sharding: row-shard W / A over 8 cores (1024 rows = 16 A-blocks each);
B replicated.  Each core computes outT = (A_bd @ W_shard @ B_bd)^T and the
host transposes each shard back on gather.

Everything that moves through HBM is bf16 (W 16 MB/core, A, B, the AW^T
intermediate, and the outT store — the host upcasts the output to f32):
~3.7e-3 rel err against the 2e-2 gate, and ~34.5 MB of HBM traffic per core,
within ~25% of the 16-engine DMA roofline for the whole kernel.

Per-core dataflow (all sizes per core):
  step 1:  T = (A_bd @ W)^T computed 128-column-chunk-wise with W as the
           matmul *stationary* operand:  matmul(lhsT=W[128 rows, 128 cols],
           rhs=blockdiag(A_even^T, A_odd^T)) -> psum [128 cols, 128 rows].
           This absorbs the transpose that a chained matmul otherwise needs.
  step 2:  outT[chunk] = matmul(lhsT=blockdiag(B_j0, B_j1), rhs=T chunk),
           software-pipelined INTO the next group's step-1 slab loop (chunk
           cc=r folded into slab r) so stores and PSUM->SBUF copies drain
           evenly instead of bursting at group boundaries.

DMA layout: W is host-retiled to [G, R, 128, 1024] so each W load is one
fully contiguous 256 KB HBM read.  A and the W tiles ride the SP HWDGE
queue (A first — it gates the first matmul); outT stores ride the queue of
whichever engine (DVE or ACT) produced the SBUF copy, so a store trigger
never waits cross-engine; bpack chunks ride the gpsimd SWDGE queue at t0
(first group) and are then paced two groups ahead on the scalar queue, so
the 2 MB preload never starves the W-load stream during warmup.
"""

import bass_rust
import numpy as np

import concourse.bass as bass
import concourse.mybir as mybir
from concourse.bass_utils import run_bass_kernel_spmd
from concourse.tile import TileContext

F32 = mybir.dt.float32
BF16 = mybir.dt.bfloat16

N_CORES = 8
D = 8192
BLK = 64
ROWS_PC = D // N_CORES  # 1024 rows of W / out per core

_HOIST_OPCODES = {"Matmult", "DMACopy", "TensorCopy", "Memset", "Activation", "Drain"}


def _hoist_excess_matmul_waits(nc: bass.Bass, max_waits: int = 1) -> None:
    """walrus's codegen for several instruction structs (fused-LDWEIGHTS
    matmul, DMA_DIRECT2D, ...) has few sync-wait slots ("Too many sync wait
    commands"). Move excess semaphore waits off such instructions into
    standalone EventSemaphore instructions right before them on the same
    engine queue — the sequencer executes those in order, so the instruction
    still starts only after all waits pass."""
    ctr = 0
    for fnc in nc.m.functions:
        for bb in fnc.blocks:
            new = []
            for ins in bb.instructions:
                si = ins.sync_info if ins.opcode in _HOIST_OPCODES else None
                if si is not None and len(si.on_wait) > max_waits:
                    waits = list(si.on_wait)
                    for w in waits[:-max_waits]:
                        evs = mybir.InstEventSemaphore(
                            name=f"mmwaithoist-{ctr}", ins=[], outs=[]
                        )
                        ctr += 1
                        evs.engine = ins.engine
                        evs.sync_info = bass_rust.SyncInfo(on_wait=[w], on_update=[])
                        new.append(evs)
                    ins.sync_info.on_wait = waits[-max_waits:]
                new.append(ins)
            bb.instructions[:] = new


def build_nc(rows_pc: int = ROWS_PC, d: int = D, hoist: bool = True) -> bass.Bass:
    """One-core SPMD program, chunk-major: each group's eight 128-col chunks
    are produced one at a time (all 8 slab matmuls per chunk back to back),
    and chunk cc's B-multiply + store self-folds at iteration cc+2 of the
    SAME group.  Stores trail the W-load stream by ~2 chunks instead of a
    whole group, so almost nothing is left to drain after the last W byte.
    hoist=False keeps waits on the original instructions (CoreSim's race
    detector wants every instruction to carry its own updates; the hoisted
    variant is for walrus, whose ISA structs have too few wait slots)."""
    R = rows_pc // 128  # 128-row slabs per core (= A-block pairs)
    G = d // 1024       # column groups of 8x128

    nc = bass.Bass()
    wq_ext = nc.declare_dram_parameter("wq", [G, R, 128, 1024], BF16, isOutput=False)
    ah_ext = nc.declare_dram_parameter("ah", [128, R * 128], BF16, isOutput=False)
    bp_ext = nc.declare_dram_parameter("bpack", [128, d], BF16, isOutput=False)
    ot_ext = nc.declare_dram_parameter("outt", [d, rows_pc], BF16, isOutput=True)

    with TileContext(nc) as tc:
        with (
            tc.tile_pool(name="const", bufs=1) as cpool,
            # 24 W bufs = 3 groups deep: the W-load stream free-runs at
            # high duty early (before stores ramp) and finishes sooner;
            # the pool's per-buf waits throttle the queue pull-based.
            tc.tile_pool(name="wp", bufs=24) as wpool,
            tc.tile_pool(name="tgc", bufs=4) as tpool,
            tc.tile_pool(name="op", bufs=6) as opool,
            tc.tile_pool(name="p1", bufs=2, space="PSUM") as p1pool,
            tc.tile_pool(name="p2", bufs=2, space="PSUM") as p2pool,
        ):
            # A rides the sync (SP) queue FIRST, just ahead of the W tiles
            # it gates; bpack rides the otherwise-idle gpsimd SWDGE queue at
            # t0 and is then paced one group ahead on the scalar queue.
            ah = cpool.tile([128, R * 128], BF16)
            nc.sync.dma_start(out=ah[:], in_=ah_ext[:])
            bptiles = [
                cpool.tile([128, 1024], BF16, name=f"bp{g}") for g in range(G)
            ]

            # The ENTIRE input stream is issued upfront on the sync queue in
            # consumption order: ah, then per group its 8 W tiles followed by
            # its bpack chunk.  Each load's buf-free wait paces the queue, so
            # loads free-run whenever engines have slack and no other queue
            # ever competes at t0.
            wt_tiles = {}
            for g in range(G):
                for r in range(R):
                    wt = wpool.tile([128, 1024], BF16, name="wt")
                    nc.sync.dma_start(out=wt[:], in_=wq_ext[g, r])
                    wt_tiles[(g, r)] = wt
                nc.sync.dma_start(
                    out=bptiles[g][:],
                    in_=bp_ext[:, g * 1024 : (g + 1) * 1024],
                )

            tg_tiles = {}

            def step2_chunk(gj: int, j: int, final: bool = False):
                """B-multiply + copy + store for chunk j of group gj.  The
                two 512-row halves go through separate 1-bank PSUM tiles;
                both copies run on ONE engine (the one opposite the chunk's
                step-1 copy) and the store triggers from that engine's queue,
                so it never waits cross-engine.  final=True splits the copies
                across both engines to cut drain latency."""
                tgc = tg_tiles.pop((gj, j))
                lb = bptiles[gj][:, j * 128 : (j + 1) * 128]
                j2 = 8 * gj + j
                ot = opool.tile([128, rows_pc], BF16, name="ot")
                act_side = j % 2 == 0
                p2 = p2pool.tile([128, rows_pc], F32, name="p2")
                for s in range(2):
                    nc.tensor.matmul(
                        p2[:, s * 512 : (s + 1) * 512],
                        lhsT=lb, rhs=tgc[:, s * 512 : (s + 1) * 512],
                        start=True, stop=True,
                    )
                if final:
                    # drain: split the copy across both engines for latency
                    nc.scalar.copy(ot[:, 0:512], p2[:, 0:512])
                    nc.vector.tensor_copy(ot[:, 512:1024], p2[:, 512:1024])
                elif act_side:
                    nc.scalar.copy(ot[:], p2[:])
                else:
                    nc.vector.tensor_copy(ot[:], p2[:])
                if act_side or final:
                    nc.scalar.dma_start(
                        out=ot_ext[j2 * 128 : (j2 + 1) * 128, :], in_=ot[:]
                    )
                else:
                    nc.gpsimd.dma_start(
                        out=ot_ext[j2 * 128 : (j2 + 1) * 128, :], in_=ot[:]
                    )

            for g in range(G):
                for cc in range(8):
                    p1 = p1pool.tile([128, rows_pc], F32, name="p1")
                    for r in range(R):
                        rs = slice(r * 128, (r + 1) * 128)
                        nc.tensor.matmul(
                            p1[:, rs],
                            lhsT=wt_tiles[(g, r)][:, cc * 128 : (cc + 1) * 128],
                            rhs=ah[:, rs],
                            start=True, stop=True,
                        )
                    # T chunk: col = 1024*g + 128*cc + p, row = free index.
                    tgc = tpool.tile([128, rows_pc], BF16, name="tgc")
                    if cc % 2 == 0:
                        nc.vector.tensor_copy(tgc[:], p1[:])
                    else:
                        nc.scalar.copy(tgc[:], p1[:])
                    tg_tiles[(g, cc)] = tgc
                    # self-fold: chunk cc-2 of this group (its tgc copy has
                    # had 2 iterations to land); the first two iterations
                    # finish the previous group's last two chunks.
                    if cc >= 2:
                        step2_chunk(g, cc - 2)
                    elif g > 0:
                        step2_chunk(g - 1, 6 + cc)
            step2_chunk(G - 1, 6, final=True)
            step2_chunk(G - 1, 7, final=True)
    if hoist:
        _hoist_excess_matmul_waits(nc)
    return nc


def pack_at(a_blocks: np.ndarray) -> np.ndarray:
    """[2R, 64, 64] A blocks -> [128, R*128] with
    out[64*b + k, 128*r + 64*b + n] = A[2r+b][n, k] (transposed, pair-blockdiag)."""
    nb = a_blocks.shape[0]
    R = nb // 2
    out = np.zeros((128, R * 128), np.float32)
    at = a_blocks.transpose(0, 2, 1)
    out[0:64].reshape(64, R, 2, 64)[:, :, 0, :] = at[0::2].transpose(1, 0, 2)
    out[64:128].reshape(64, R, 2, 64)[:, :, 1, :] = at[1::2].transpose(1, 0, 2)
    return out


def pack_b(b_blocks: np.ndarray) -> np.ndarray:
    """[2J, 64, 64] B blocks -> [128, J*128] with
    out[64*b + k, 128*j + 64*b + n] = B[2j+b][k, n] (pair-blockdiag, untransposed)."""
    nb = b_blocks.shape[0]
    J = nb // 2
    out = np.zeros((128, J * 128), np.float32)
    out[0:64].reshape(64, J, 2, 64)[:, :, 0, :] = b_blocks[0::2].transpose(1, 0, 2)
    out[64:128].reshape(64, J, 2, 64)[:, :, 1, :] = b_blocks[1::2].transpose(1, 0, 2)
    return out


def pack_w(w_shard: np.ndarray):
    """[rows_pc, d] -> bf16 [G, R, 128, 1024] so each (g, r) W tile is one
    contiguous 256 KB block in DRAM."""
    import ml_dtypes

    rows_pc, d = w_shard.shape
    R, G = rows_pc // 128, d // 1024
    wt = w_shard.reshape(R, 128, G, 1024).transpose(2, 0, 1, 3)
    return np.ascontiguousarray(wt.astype(ml_dtypes.bfloat16))


_NC_CACHE: dict = {}


def run(W, A_blocks, B_blocks, trace: bool = False, trace_cores=None):
    W = np.asarray(W, dtype=np.float32)
    A_blocks = np.asarray(A_blocks, dtype=np.float32)
    B_blocks = np.asarray(B_blocks, dtype=np.float32)
    assert W.shape == (D, D) and A_blocks.shape == (D // BLK, BLK, BLK)

    if "nc" not in _NC_CACHE:
        _NC_CACHE["nc"] = build_nc()
    nc = _NC_CACHE["nc"]

    import ml_dtypes

    bp = pack_b(B_blocks).astype(ml_dtypes.bfloat16)
    in_maps = []
    for c in range(N_CORES):
        wq = pack_w(W[ROWS_PC * c : ROWS_PC * (c + 1)])
        ah = pack_at(A_blocks[16 * c : 16 * (c + 1)]).astype(ml_dtypes.bfloat16)
        in_maps.append({"wq": wq, "ah": ah, "bpack": bp})
    res = run_bass_kernel_spmd(nc, in_maps, core_ids=list(range(N_CORES)), trace=trace, trace_cores=trace_cores)
    out = np.empty((D, D), np.float32)
    for c in range(N_CORES):
        out[ROWS_PC * c : ROWS_PC * (c + 1), :] = res.results[c]["outt"].T.astype(
            np.float32
        )
    return out, res


def kernel(W, A_blocks, B_blocks):
    out, _ = run(W, A_blocks, B_blocks, trace=False)
    return out


# revision 40
# speedup vs baseline: 1.0969x; 1.0969x over previous
"""Block-diagonal compress kernel: out = blockdiag(A) @ W @ blockdiag(B).

Shapes (full): W [8192, 8192] f32, A_blocks [128, 64, 64], B_blocks [128, 64, 64].
Sharding: row-shard W / A over 8 cores (1024 rows = 16 A-blocks each);
B replicated.  Each core computes outT = (A_bd @ W_shard @ B_bd)^T and the
host transposes each shard back on gather.

Everything that moves through HBM is bf16 (W 16 MB/core, A, B, the AW^T
intermediate, and the outT store — the host upcasts the output to f32):
~3.7e-3 rel err against the 2e-2 gate, and ~34.5 MB of HBM traffic per core,
within ~25% of the 16-engine DMA roofline for the whole kernel.

Per-core dataflow (all sizes per core):
  step 1:  T = (A_bd @ W)^T computed 128-column-chunk-wise with W as the
           matmul *stationary* operand:  matmul(lhsT=W[128 rows, 128 cols],
           rhs=blockdiag(A_even^T, A_odd^T)) -> psum [128 cols, 128 rows].
           This absorbs the transpose that a chained matmul otherwise needs.
  step 2:  outT[chunk] = matmul(lhsT=blockdiag(B_j0, B_j1), rhs=T chunk),
           software-pipelined INTO the next group's step-1 slab loop (chunk
           cc=r folded into slab r) so stores and PSUM->SBUF copies drain
           evenly instead of bursting at group boundaries.

DMA layout: W is host-retiled to [G, R, 128, 1024] so each W load is one
fully contiguous 256 KB HBM read.  A and the W tiles ride the SP HWDGE
queue (A first — it gates the first matmul); outT stores ride the queue of
whichever engine (DVE or ACT) produced the SBUF copy, so a store trigger
never waits cross-engine; bpack chunks ride the gpsimd SWDGE queue at t0
(first group) and are then paced two groups ahead on the scalar queue, so
the 2 MB preload never starves the W-load stream during warmup.
"""

import bass_rust
import numpy as np

import concourse.bass as bass
import concourse.mybir as mybir
from concourse.bass_utils import run_bass_kernel_spmd
from concourse.tile import TileContext

F32 = mybir.dt.float32
BF16 = mybir.dt.bfloat16

N_CORES = 8
D = 8192
BLK = 64
ROWS_PC = D // N_CORES  # 1024 rows of W / out per core

_HOIST_OPCODES = {"Matmult", "DMACopy", "TensorCopy", "Memset", "Activation", "Drain"}


def _hoist_excess_matmul_waits(nc: bass.Bass, max_waits: int = 1) -> None:
    """walrus's codegen for several instruction structs (fused-LDWEIGHTS
    matmul, DMA_DIRECT2D, ...) has few sync-wait slots ("Too many sync wait
    commands"). Move excess semaphore waits off such instructions into
    standalone EventSemaphore instructions right before them on the same
    engine queue — the sequencer executes those in order, so the instruction
    still starts only after all waits pass."""
    ctr = 0
    for fnc in nc.m.functions:
        for bb in fnc.blocks:
            new = []
            for ins in bb.instructions:
                si = ins.sync_info if ins.opcode in _HOIST_OPCODES else None
                if si is not None and len(si.on_wait) > max_waits:
                    waits = list(si.on_wait)
                    for w in waits[:-max_waits]:
                        evs = mybir.InstEventSemaphore(
                            name=f"mmwaithoist-{ctr}", ins=[], outs=[]
                        )
                        ctr += 1
                        evs.engine = ins.engine
                        evs.sync_info = bass_rust.SyncInfo(on_wait=[w], on_update=[])
                        new.append(evs)
                    ins.sync_info.on_wait = waits[-max_waits:]
                new.append(ins)
            bb.instructions[:] = new


def build_nc(rows_pc: int = ROWS_PC, d: int = D, hoist: bool = True) -> bass.Bass:
    """One-core SPMD program, chunk-major: each group's eight 128-col chunks
    are produced one at a time (all 8 slab matmuls per chunk back to back),
    and chunk cc's B-multiply + store self-folds at iteration cc+2 of the
    SAME group.  Stores trail the W-load stream by ~2 chunks instead of a
    whole group, so almost nothing is left to drain after the last W byte.
    hoist=False keeps waits on the original instructions (CoreSim's race
    detector wants every instruction to carry its own updates; the hoisted
    variant is for walrus, whose ISA structs have too few wait slots)."""
    R = rows_pc // 128  # 128-row slabs per core (= A-block pairs)
    G = d // 1024       # column groups of 8x128

    nc = bass.Bass()
    wq_ext = nc.declare_dram_parameter("wq", [G, R, 128, 1024], BF16, isOutput=False)
    ah_ext = nc.declare_dram_parameter("ah", [128, R * 128], BF16, isOutput=False)
    bp_ext = nc.declare_dram_parameter("bpack", [128, d], BF16, isOutput=False)
    ot_ext = nc.declare_dram_parameter("outt", [d, rows_pc], BF16, isOutput=True)

    with TileContext(nc) as tc:
        with (
            tc.tile_pool(name="const", bufs=1) as cpool,
            # 16 W bufs: the full current group (all 8 tiles stay live for
            # every chunk) plus the next group prefetching behind it.  One
            # group of lookahead rate-matches loads to compute: deeper
            # prefetch measurably thrashes HBM (+17% per-byte) and piles
            # stores into the tail.
            tc.tile_pool(name="wp", bufs=16) as wpool,
            tc.tile_pool(name="tgc", bufs=4) as tpool,
            tc.tile_pool(name="op", bufs=6) as opool,
            tc.tile_pool(name="p1", bufs=2, space="PSUM") as p1pool,
            tc.tile_pool(name="p2", bufs=2, space="PSUM") as p2pool,
        ):
            # A rides the sync (SP) queue FIRST, just ahead of the W tiles
            # it gates; bpack rides the otherwise-idle gpsimd SWDGE queue at
            # t0 and is then paced one group ahead on the scalar queue.
            ah = cpool.tile([128, R * 128], BF16)
            nc.sync.dma_start(out=ah[:], in_=ah_ext[:])
            bptiles = [
                cpool.tile([128, 1024], BF16, name=f"bp{g}") for g in range(G)
            ]

            wt_tiles = {}
            for r in range(R):
                wt = wpool.tile([128, 1024], BF16, name="wt")
                nc.sync.dma_start(out=wt[:], in_=wq_ext[0, r])
                wt_tiles[(0, r)] = wt
            # bp0 rides the sync queue BEHIND the first group's W tiles:
            # it lands just-in-time for chunk 0's fold with zero t0
            # contention against the W stream.
            nc.sync.dma_start(out=bptiles[0][:], in_=bp_ext[:, 0:1024])

            tg_tiles = {}

            def step2_chunk(gj: int, j: int, final: bool = False):
                """B-multiply + copy + store for chunk j of group gj.  The
                two 512-row halves go through separate 1-bank PSUM tiles;
                both copies run on ONE engine (the one opposite the chunk's
                step-1 copy) and the store triggers from that engine's queue,
                so it never waits cross-engine.  final=True splits the copies
                across both engines to cut drain latency."""
                tgc = tg_tiles.pop((gj, j))
                lb = bptiles[gj][:, j * 128 : (j + 1) * 128]
                j2 = 8 * gj + j
                ot = opool.tile([128, rows_pc], BF16, name="ot")
                act_side = j % 2 == 0
                p2 = p2pool.tile([128, rows_pc], F32, name="p2")
                for s in range(2):
                    nc.tensor.matmul(
                        p2[:, s * 512 : (s + 1) * 512],
                        lhsT=lb, rhs=tgc[:, s * 512 : (s + 1) * 512],
                        start=True, stop=True,
                    )
                if final:
                    # drain: split the copy across both engines for latency
                    nc.scalar.copy(ot[:, 0:512], p2[:, 0:512])
                    nc.vector.tensor_copy(ot[:, 512:1024], p2[:, 512:1024])
                elif act_side:
                    nc.scalar.copy(ot[:], p2[:])
                else:
                    nc.vector.tensor_copy(ot[:], p2[:])
                if act_side or final:
                    nc.scalar.dma_start(
                        out=ot_ext[j2 * 128 : (j2 + 1) * 128, :], in_=ot[:]
                    )
                else:
                    nc.gpsimd.dma_start(
                        out=ot_ext[j2 * 128 : (j2 + 1) * 128, :], in_=ot[:]
                    )

            for g in range(G):
                for cc in range(8):
                    if g + 1 < G:
                        # prefetch one slab of the NEXT group per iteration
                        wt = wpool.tile([128, 1024], BF16, name="wt")
                        nc.sync.dma_start(out=wt[:], in_=wq_ext[g + 1, cc])
                        wt_tiles[(g + 1, cc)] = wt
                    p1 = p1pool.tile([128, rows_pc], F32, name="p1")
                    for r in range(R):
                        rs = slice(r * 128, (r + 1) * 128)
                        nc.tensor.matmul(
                            p1[:, rs],
                            lhsT=wt_tiles[(g, r)][:, cc * 128 : (cc + 1) * 128],
                            rhs=ah[:, rs],
                            start=True, stop=True,
                        )
                    # T chunk: col = 1024*g + 128*cc + p, row = free index.
                    tgc = tpool.tile([128, rows_pc], BF16, name="tgc")
                    if cc % 2 == 0:
                        nc.vector.tensor_copy(tgc[:], p1[:])
                    else:
                        nc.scalar.copy(tgc[:], p1[:])
                    tg_tiles[(g, cc)] = tgc
                    if cc == 4 and g + 1 < G:
                        nc.scalar.dma_start(
                            out=bptiles[g + 1][:],
                            in_=bp_ext[:, (g + 1) * 1024 : (g + 2) * 1024],
                        )
                    # self-fold: chunk cc-2 of this group (its tgc copy has
                    # had 2 iterations to land); the first two iterations
                    # finish the previous group's last two chunks.
                    if cc >= 2:
                        step2_chunk(g, cc - 2)
                    elif g > 0:
                        step2_chunk(g - 1, 6 + cc)
            step2_chunk(G - 1, 6, final=True)
            step2_chunk(G - 1, 7, final=True)
    if hoist:
        _hoist_excess_matmul_waits(nc)
    return nc


def pack_at(a_blocks: np.ndarray) -> np.ndarray:
    """[2R, 64, 64] A blocks -> [128, R*128] with
    out[64*b + k, 128*r + 64*b + n] = A[2r+b][n, k] (transposed, pair-blockdiag)."""
    nb = a_blocks.shape[0]
    R = nb // 2
    out = np.zeros((128, R * 128), np.float32)
    at = a_blocks.transpose(0, 2, 1)
    out[0:64].reshape(64, R, 2, 64)[:, :, 0, :] = at[0::2].transpose(1, 0, 2)
    out[64:128].reshape(64, R, 2, 64)[:, :, 1, :] = at[1::2].transpose(1, 0, 2)
    return out


def pack_b(b_blocks: np.ndarray) -> np.ndarray:
    """[2J, 64, 64] B blocks -> [128, J*128] with
    out[64*b + k, 128*j + 64*b + n] = B[2j+b][k, n] (pair-blockdiag, untransposed)."""
    nb = b_blocks.shape[0]
    J = nb // 2
    out = np.zeros((128, J * 128), np.float32)
    out[0:64].reshape(64, J, 2, 64)[:, :, 0, :] = b_blocks[0::2].transpose(1, 0, 2)
    out[64:128].reshape(64, J, 2, 64)[:, :, 1, :] = b_blocks[1::2].transpose(1, 0, 2)
    return out


def pack_w(w_shard: np.ndarray):
    """[rows_pc, d] -> bf16 [G, R, 128, 1024] so each (g, r) W tile is one
    contiguous 256 KB block in DRAM."""
    import ml_dtypes

    rows_pc, d = w_shard.shape
    R, G = rows_pc // 128, d // 1024
    wt = w_shard.reshape(R, 128, G, 1024).transpose(2, 0, 1, 3)
    return np.ascontiguousarray(wt.astype(ml_dtypes.bfloat16))


_NC_CACHE: dict = {}


def run(W, A_blocks, B_blocks, trace: bool = False, trace_cores=None):
    W = np.asarray(W, dtype=np.float32)
    A_blocks = np.asarray(A_blocks, dtype=np.float32)
    B_blocks = np.asarray(B_blocks, dtype=np.float32)
    assert W.shape == (D, D) and A_blocks.shape == (D // BLK, BLK, BLK)

    if "nc" not in _NC_CACHE:
        _NC_CACHE["nc"] = build_nc()
    nc = _NC_CACHE["nc"]

    import ml_dtypes

    bp = pack_b(B_blocks).astype(ml_dtypes.bfloat16)
    in_maps = []
    for c in range(N_CORES):
        wq = pack_w(W[ROWS_PC * c : ROWS_PC * (c + 1)])
        ah = pack_at(A_blocks[16 * c : 16 * (c + 1)]).astype(ml_dtypes.bfloat16)
        in_maps.append({"wq": wq, "ah": ah, "bpack": bp})
    res = run_bass_kernel_spmd(nc, in_maps, core_ids=list(range(N_CORES)), trace=trace, trace_cores=trace_cores)
    out = np.empty((D, D), np.float32)
    for c in range(N_CORES):
        out[ROWS_PC * c : ROWS_PC * (c + 1), :] = res.results[c]["outt"].T.astype(
            np.float32
        )
    return out, res


def kernel(W, A_blocks, B_blocks):
    out, _ = run(W, A_blocks, B_blocks, trace=False)
    return out
